# revision 1
# baseline (speedup 1.0000x reference)
"""Trainium2 Bass kernel for nn_C2BM_30537217474758 (gnn_message_passing).

Concept-bottleneck model:
  x_enc = lrelu(x @ W_enc + b_enc)                         [B, 1024]
  vals  = lrelu(einsum('bi,rio->bro', x_enc, Wv) + bv)     [B, 8, 256]
  p_root = softmax(einsum('bro,roc->brc', vals, Ws) + bs)  [B, 8, 4]
  p_root = intervene(p_root, c[:, :8], ii[:, :8])
  h     = lrelu(einsum('bp,nph->bnh', p_root.flat, W1c) + b1c)
  p_mid = softmax(einsum('bnh,nhc->bnc', h, W2c) + b2c); intervene
  y     = softmax(lrelu(p_mid.flat @ W1y + b1y) @ W2y + b2y)
  out   = concat([p_root, p_mid, y[:, None]], axis=1)      [B, 17, 4]

Strategy: pure data-parallel over 8 NeuronCores (batch shard 1024/core),
weights replicated. Channels live on SBUF partitions and batch on the free
dimension for the two large GEMMs (contraction on partitions); x is
transposed on-chip via the PE (bf16 cast during the SWDGE DMA load).
Matmuls run in bf16 with fp32 PSUM accumulation; softmax / intervention
arithmetic is fp32. The batch is processed in two 512-row halves so each
half's softmax->propagator->task tail (DVE/ACT latency chains) hides under
the other half's GEMMs. Small biases are folded into the matmuls via
ones-row augmented operands.
"""

import os
import sys

try:
    import concourse  # noqa: F401
except ImportError:
    sys.path.insert(0, "/opt/trn_rl_repo")

import numpy as np
import ml_dtypes

import concourse.bacc as bacc
import concourse.tile as tile
from concourse import mybir

# ---------------- problem constants (hardcoded per contract) ----------------
B, D_IN, D_H = 8192, 2048, 1024
N_ROOT, N_MID, CARD, CHS = 8, 8, 4, 64
OV = CARD * CHS           # 256  value-embedding width per root
P_IN = N_ROOT * CARD      # 32
P_HID = 2 * P_IN          # 64
N_CORES = 8
BSH = B // N_CORES        # 1024 batch rows per core
NBT = BSH // 128          # 8 partition-tiles of batch
KT_IN = D_IN // 128       # 16 contraction tiles for encoder
KT_H = D_H // 128         # 8 contraction tiles for Wv
OUTW = 17 * CARD          # 68 output cols per row

F32 = mybir.dt.float32
I32 = mybir.dt.int32
BF16 = mybir.dt.bfloat16
AF = mybir.ActivationFunctionType
ALU = mybir.AluOpType
AX = mybir.AxisListType

MM_DTYPE = os.environ.get("BASS_MM_DTYPE", "bf16")  # bf16 | fp32r | fp32
LRELU_ALPHA = 0.01
# CoreSim does not implement Lrelu; BASS_SIM_SAFE=1 swaps in Relu so the
# rest of the program can be validated in simulation.
SIM_SAFE = os.environ.get("BASS_SIM_SAFE") == "1"
ACT_LRELU = AF.Relu if SIM_SAFE else AF.Lrelu


def _mmdt():
    return BF16 if MM_DTYPE == "bf16" else F32


def _mm(ap):
    """View an AP in the matmul dtype (fp32r bitcast when requested)."""
    if MM_DTYPE == "fp32r":
        return ap.bitcast(mybir.dt.float32r)
    return ap


def build_program():
    """Emit the per-core Bass program (identical on all 8 cores)."""
    nc = bacc.Bacc("TRN2", target_bir_lowering=False, debug=False,
                   num_devices=N_CORES)
    mdt = _mmdt()

    # ------------- DRAM I/O -------------
    x_d = nc.dram_tensor("x", [BSH, D_IN], F32, kind="ExternalInput")
    lab_d = nc.dram_tensor("lab", [BSH, 17], I32, kind="ExternalInput")
    msk_d = nc.dram_tensor("msk", [BSH, 17], I32, kind="ExternalInput")
    wenc_d = nc.dram_tensor("wenc", [D_IN, D_H], mdt, kind="ExternalInput")
    wv_d = nc.dram_tensor("wv", [D_H, N_ROOT * OV], mdt, kind="ExternalInput")
    ws_d = nc.dram_tensor("ws", [OV, N_ROOT * CARD], mdt, kind="ExternalInput")
    # [33, 4, 128]: rows 0-31 W1c mid-pairs, row 32 = b1c (ones-row fold)
    w1c_d = nc.dram_tensor("w1c", [P_IN + 1, 4 * 128], mdt, kind="ExternalInput")
    w2c_d = nc.dram_tensor("w2c", [128, 4 * 8], mdt, kind="ExternalInput")
    w1y_d = nc.dram_tensor("w1y", [P_IN + 1, P_HID], mdt, kind="ExternalInput")
    w2y_d = nc.dram_tensor("w2y", [P_HID + 1, CARD], mdt, kind="ExternalInput")
    benc_d = nc.dram_tensor("benc", [128, KT_H], F32, kind="ExternalInput")
    bv_d = nc.dram_tensor("bv", [128, 16], F32, kind="ExternalInput")
    bsr_d = nc.dram_tensor("bsr", [1, 32], mdt, kind="ExternalInput")
    b2cr_d = nc.dram_tensor("b2cr", [1, 32], mdt, kind="ExternalInput")
    ident_d = nc.dram_tensor("ident", [128, 128], mdt, kind="ExternalInput")
    identf_d = nc.dram_tensor("identf", [128, 128], F32, kind="ExternalInput")
    iotaf_d = nc.dram_tensor("iotaf", [128, 32], F32, kind="ExternalInput")
    out_d = nc.dram_tensor("out", [BSH, OUTW], F32, kind="ExternalOutput")

    with tile.TileContext(nc) as tc:
        with (
            tc.tile_pool(name="persist", bufs=1) as persist,
            tc.tile_pool(name="xraw", bufs=5) as xraw_pool,
            tc.tile_pool(name="vals", bufs=2) as vals_pool,
            tc.tile_pool(name="stage", bufs=3) as stage_pool,
            tc.tile_pool(name="tmp32", bufs=4) as tmp32_pool,
            tc.tile_pool(name="tmp8", bufs=6) as tmp8_pool,
            tc.tile_pool(name="outp", bufs=2) as out_pool,
            tc.tile_pool(name="ps_mm", bufs=4, space="PSUM") as ps_mm,
            tc.tile_pool(name="ps_lg", bufs=2, space="PSUM") as ps_lg,
            tc.tile_pool(name="ps_tr", bufs=2, space="PSUM") as ps_tr,
        ):
            ident_sb = persist.tile([128, 128], mdt)
            nc.sync.dma_start(out=ident_sb, in_=ident_d.ap())
            identf_sb = persist.tile([128, 128], F32)
            nc.sync.dma_start(out=identf_sb, in_=identf_d.ap())
            iotaf_sb = persist.tile([128, 32], F32)
            nc.sync.dma_start(out=iotaf_sb, in_=iotaf_d.ap())

            # -------- x pipeline + big weights first (gates the encoder) ----
            # x: SWDGE loads with f32->bf16 cast (own ring; W_enc streams on
            # the SP ring concurrently), then 128x128 PE transposes with DVE
            # draining PSUM. The xbar DMA-transpose path is avoided: it
            # serializes against every other HWDGE DMA copy.
            wenc_sb = persist.tile([128, KT_IN, D_H], mdt)
            wenc_r = wenc_d.ap().rearrange("(kt p) h -> p kt h", p=128)
            xt_sb = persist.tile([128, KT_IN, BSH], mdt)  # xT: [d, b]
            xbs = []
            for bt in range(NBT):
                src = x_d.ap()[bt * 128:(bt + 1) * 128, :]
                xb = xraw_pool.tile([128, D_IN], mdt, tag="xb", bufs=4)
                nc.gpsimd.dma_start(out=xb, in_=src)  # SWDGE casts f32->bf16
                xbs.append(xb)
                # all of W_enc rides with x bt0-3 (the encoder's first half
                # only needs those); x bt4-7 follows via the xb pool slots
                if bt < 4:
                    for kt in range(4 * bt, 4 * bt + 4):
                        nc.sync.dma_start(out=wenc_sb[:, kt, :],
                                          in_=wenc_r[:, kt, :])

            def transpose_bt(bt):
                xb = xbs[bt]
                for kt in range(KT_IN):
                    trp = ps_tr.tile([128, 128], mdt, tag="ptr")
                    nc.tensor.transpose(trp, _mm(xb[:, kt * 128:(kt + 1) * 128]),
                                        _mm(ident_sb))
                    nc.vector.tensor_copy(
                        xt_sb[:, kt, bt * 128:(bt + 1) * 128], trp)

            # kt-major for bt0-3: the encoder accumulates k sequentially,
            # so finishing all four batch-tiles of k-tile j unblocks the
            # encoder's j-th matmul step while later k-tiles still transpose
            for kt in range(KT_IN):
                for bt in range(4):
                    trp = ps_tr.tile([128, 128], mdt, tag="ptr")
                    nc.tensor.transpose(
                        trp, _mm(xbs[bt][:, kt * 128:(kt + 1) * 128]),
                        _mm(ident_sb))
                    nc.vector.tensor_copy(
                        xt_sb[:, kt, bt * 128:(bt + 1) * 128], trp)

            wv_sb = persist.tile([128, KT_H, N_ROOT * OV], mdt)
            wv_r = wv_d.ap().rearrange("(kt p) o -> p kt o", p=128)
            for kt in range(KT_H):
                nc.sync.dma_start(out=wv_sb[:, kt, :], in_=wv_r[:, kt, :])
            lab_sb = persist.tile([128, NBT, 17], I32)
            nc.sync.dma_start(out=lab_sb,
                              in_=lab_d.ap().rearrange("(t p) k -> p t k", p=128))
            msk_sb = persist.tile([128, NBT, 17], I32)
            nc.sync.dma_start(out=msk_sb,
                              in_=msk_d.ap().rearrange("(t p) k -> p t k", p=128))

            ws_sb = persist.tile([128, 2, 32], mdt)
            nc.sync.dma_start(out=ws_sb,
                              in_=ws_d.ap().rearrange("(kt p) c -> p kt c", p=128))
            w1c_sb = persist.tile([P_IN + 1, 4, 128], mdt)
            nc.sync.dma_start(out=w1c_sb,
                              in_=w1c_d.ap().rearrange("p (q m) -> p q m", m=128))
            w2c_sb = persist.tile([128, 4, 8], mdt)
            nc.sync.dma_start(out=w2c_sb,
                              in_=w2c_d.ap().rearrange("p (q c) -> p q c", c=8))
            w1y_sb = persist.tile([P_IN + 1, P_HID], mdt)
            nc.sync.dma_start(out=w1y_sb, in_=w1y_d.ap())
            w2y_sb = persist.tile([P_HID + 1, CARD], mdt)
            nc.sync.dma_start(out=w2y_sb, in_=w2y_d.ap())
            benc_sb = persist.tile([128, KT_H], F32)
            nc.sync.dma_start(out=benc_sb, in_=benc_d.ap())
            bv_sb = persist.tile([128, 16], F32)
            nc.sync.dma_start(out=bv_sb, in_=bv_d.ap())
            bsr_sb = persist.tile([1, 32], mdt)
            nc.sync.dma_start(out=bsr_sb, in_=bsr_d.ap())
            b2cr_sb = persist.tile([1, 32], mdt)
            nc.sync.dma_start(out=b2cr_sb, in_=b2cr_d.ap())
            ones_sb = persist.tile([1, 128], mdt)
            nc.vector.memset(ones_sb, 1.0)

            # ---------------- persistent activations ----------------
            xenc_sb = persist.tile([128, KT_H, BSH], mdt)   # x_encT: [h, b]
            prT_sb = persist.tile([P_HID, BSH], mdt)  # [32 p | ones] x b
            pmT_sb = persist.tile([P_HID, BSH], mdt)
            hyT_sb = persist.tile([P_HID + 1, BSH], mdt)    # row 64 = ones
            nc.vector.memset(hyT_sb[P_HID:P_HID + 1, :], 1.0)
            hT_sb = persist.tile([128, 4, BSH], mdt)  # [2 mids x 64h, b]

            # output rows for batch-tiles 4g..4g+3, packed [128, 4*68]
            osb_gs = [out_pool.tile([128, 4 * OUTW], F32, tag="osbg",
                                    name=f"osbg{i}") for i in range(2)]

            def osb_view(g, lo, hi):
                """[128, 4, hi-lo, 4] view of output cols [lo*4, hi*4)."""
                return (osb_gs[g].rearrange("p (b k) -> p b k", k=OUTW)
                        [:, :, lo * 4:hi * 4]
                        .rearrange("p b (g c) -> p b g c", c=CARD))

            # --------- precomputed intervention one-hots and masks ----------
            # oh[g][lv] = onehot(label) as f32, m[g][lv] = mask as i32,
            # both [128, 4bt * 8grp * 4card]; they only depend on lab/msk.
            oh_t = {}
            m_t = {}

            def pview(t):
                """[128, 4bt, 8, 4] view of the 32 data cols of each 64-col
                bt-block in a [128, 256] staging tile."""
                return (t.rearrange("p (b k) -> p b k", k=P_HID)[:, :, 0:32]
                        .rearrange("p b (g c) -> p b g c", c=CARD))

            def make_ohm(g, lv):
                labf = tmp8_pool.tile([128, 32], F32, tag="labf")
                nc.vector.tensor_copy(
                    labf.rearrange("p (b g) -> p b g", b=4),
                    lab_sb[:, 4 * g:4 * g + 4, lv * 8:lv * 8 + 8])
                oh = persist.tile([128, 256], F32, name=f"oh{g}{lv}")
                nc.vector.tensor_tensor(
                    pview(oh),
                    labf.rearrange("p (b g) -> p b g", b=4)
                    .unsqueeze(3).broadcast_to([128, 4, 8, CARD]),
                    iotaf_sb.rearrange("p (g c) -> p g c", c=CARD)
                    .unsqueeze(1).broadcast_to([128, 4, 8, CARD]),
                    op=ALU.is_equal)
                m = persist.tile([128, 256], I32, name=f"m{g}{lv}")
                nc.vector.tensor_copy(
                    pview(m),
                    msk_sb[:, 4 * g:4 * g + 4, lv * 8:lv * 8 + 8]
                    .unsqueeze(3).broadcast_to([128, 4, 8, CARD]))
                oh_t[(g, lv)] = oh
                m_t[(g, lv)] = m

            # ---------------- encoder GEMM -> x_encT ----------------
            def encoder_half(bh):
                for ht in range(KT_H):
                    ps = ps_mm.tile([128, 512], F32, tag="mm")
                    for kt in range(KT_IN):
                        nc.tensor.matmul(
                            ps,
                            _mm(wenc_sb[:, kt, ht * 128:(ht + 1) * 128]),
                            _mm(xt_sb[:, kt, bh * 512:(bh + 1) * 512]),
                            start=(kt == 0), stop=(kt == KT_IN - 1))
                    nc.scalar.activation(
                        xenc_sb[:, ht, bh * 512:(bh + 1) * 512], ps,
                        ACT_LRELU, bias=benc_sb[:, ht:ht + 1], scale=1.0,
                        alpha=LRELU_ALPHA)

            # ------------- per-root value GEMM + scorer (one half) ----------
            def vals_scorer_half(g, lg, extra_pe=None):
                """Value embeddings + root scorer for batch rows
                [512g, 512(g+1)); logits into lg [128, 4bt x 32].
                extra_pe: dict {r: fn} emitting extra PE work after root r
                (used to sprinkle transposes / the other half's tail)."""
                # bias row first: opens each [128,32] region with start=True,
                # scorer matmuls then accumulate onto it
                for bti in range(4):
                    nc.tensor.matmul(
                        lg[:, bti * 32:(bti + 1) * 32], _mm(ones_sb),
                        _mm(bsr_sb), start=True, stop=False,
                        skip_group_check=True)
                for r in range(N_ROOT):
                    vals_sb = vals_pool.tile([128, 2, 512], mdt, tag="vals")
                    for ot in range(2):
                        ps = ps_mm.tile([128, 512], F32, tag="mm")
                        for kt in range(KT_H):
                            nc.tensor.matmul(
                                ps,
                                _mm(wv_sb[:, kt, r * OV + ot * 128:
                                          r * OV + (ot + 1) * 128]),
                                _mm(xenc_sb[:, kt, g * 512:(g + 1) * 512]),
                                start=(kt == 0), stop=(kt == KT_H - 1))
                        nc.scalar.activation(
                            vals_sb[:, ot, :], ps, ACT_LRELU,
                            bias=bv_sb[:, 2 * r + ot:2 * r + ot + 1],
                            scale=1.0, alpha=LRELU_ALPHA)
                    for bti in range(4):
                        dst = lg[:, bti * 32 + r * 4:bti * 32 + r * 4 + 4]
                        for kt in range(2):
                            nc.tensor.matmul(
                                dst,
                                _mm(vals_sb[:, kt, bti * 128:(bti + 1) * 128]),
                                _mm(ws_sb[:, kt, r * 4:(r + 1) * 4]),
                                start=False, stop=(kt == 1),
                                skip_group_check=True)
                    if extra_pe and r in extra_pe:
                        extra_pe[r]()

            # ---------------- tail stages for one half ----------------
            def softmax_chain(g, lg, lv):
                """exp/softmax + intervention on [128, 4bt x 32] logits;
                probs -> osb_gs[g] and pfin (contiguous). Returns pfin."""
                e = tmp32_pool.tile([128, 128], F32, tag="e")
                nc.scalar.activation(e, lg, AF.Exp)
                s = tmp8_pool.tile([128, 32], F32, tag="s")
                nc.vector.reduce_sum(s, e.rearrange("p (x c) -> p x c", c=CARD),
                                     axis=AX.X)
                rcp = tmp8_pool.tile([128, 32], F32, tag="rcp")
                nc.vector.reciprocal(rcp, s)
                pfin = tmp32_pool.tile([128, 256], F32, tag="pfin")
                nc.vector.memset(
                    pfin.rearrange("p (b k) -> p b k", k=P_HID)[:, :, 32:P_HID],
                    1.0)
                nc.vector.tensor_tensor(
                    pview(pfin),
                    e.rearrange("p (b g c) -> p b g c", b=4, c=CARD),
                    rcp.rearrange("p (b g) -> p b g", b=4)
                    .unsqueeze(3).broadcast_to([128, 4, 8, CARD]),
                    op=ALU.mult)
                nc.vector.copy_predicated(pview(pfin), pview(m_t[(g, lv)]),
                                          pview(oh_t[(g, lv)]))
                return pfin

            def osb_store(g, pfin, lv):
                nc.vector.tensor_copy(osb_view(g, lv * 8, lv * 8 + 8),
                                      pview(pfin))

            def p_transposes(g, pfin, pT_dst):
                """pfin [128, 4bt x (32 probs | 32 ones)] -> pT_dst
                [0:32 probs | ones rows, batch cols] (bf16 cast on drain)."""
                for bti in range(4):
                    bt = 4 * g + bti
                    trp = ps_tr.tile([P_HID, 128], F32, tag="ptr")
                    nc.tensor.transpose(
                        trp, pfin[:, bti * P_HID:(bti + 1) * P_HID], identf_sb)
                    nc.vector.tensor_copy(
                        pT_dst[:, bt * 128:(bt + 1) * 128], trp)

            def mid_h_mms(g):
                for q in range(4):
                    ps = ps_mm.tile([128, 512], F32, tag="mm")
                    nc.tensor.matmul(
                        ps, _mm(w1c_sb[:, q, :]),
                        _mm(prT_sb[0:P_IN + 1, g * 512:(g + 1) * 512]),
                        start=True, stop=True)
                    nc.scalar.activation(
                        hT_sb[:, q, g * 512:(g + 1) * 512], ps,
                        ACT_LRELU, alpha=LRELU_ALPHA)

            def mid_logit_mms(g, ml):
                for bti in range(4):
                    bt = 4 * g + bti
                    nc.tensor.matmul(
                        ml[:, bti * 32:(bti + 1) * 32], _mm(ones_sb),
                        _mm(b2cr_sb), start=True, stop=False,
                        skip_group_check=True)
                    for q in range(4):
                        nc.tensor.matmul(
                            ml[:, bti * 32 + q * 8:bti * 32 + (q + 1) * 8],
                            _mm(hT_sb[:, q, bt * 128:(bt + 1) * 128]),
                            _mm(w2c_sb[:, q, :]),
                            start=False, stop=True,
                            skip_group_check=True)

            def task_mms(g, yl):
                ps = ps_mm.tile([P_HID, 512], F32, tag="mm")
                nc.tensor.matmul(
                    ps, _mm(w1y_sb),
                    _mm(pmT_sb[0:P_IN + 1, g * 512:(g + 1) * 512]),
                    start=True, stop=True)
                hyf = stage_pool.tile([P_HID, 512], F32, tag="hyf")
                nc.vector.tensor_copy(hyf, ps)
                if SIM_SAFE:
                    nc.vector.tensor_scalar(
                        hyT_sb[0:P_HID, g * 512:(g + 1) * 512], hyf,
                        0.0, None, op0=ALU.max)
                else:
                    nc.vector.scalar_tensor_tensor(
                        hyT_sb[0:P_HID, g * 512:(g + 1) * 512], hyf,
                        LRELU_ALPHA, hyf, op0=ALU.mult, op1=ALU.max)
                for bti in range(4):
                    bt = 4 * g + bti
                    nc.tensor.matmul(
                        yl[:, bti * 4:(bti + 1) * 4],
                        _mm(hyT_sb[:, bt * 128:(bt + 1) * 128]), _mm(w2y_sb),
                        start=True, stop=True)

            def y_tail(g, yl):
                e4 = tmp8_pool.tile([128, 16], F32, tag="e4")
                nc.scalar.activation(e4, yl, AF.Exp)
                s1 = tmp8_pool.tile([128, 4], F32, tag="s1")
                nc.vector.reduce_sum(
                    s1, e4.rearrange("p (b c) -> p b c", c=CARD), axis=AX.X)
                r1 = tmp8_pool.tile([128, 4], F32, tag="r1")
                nc.vector.reciprocal(r1, s1)
                nc.vector.tensor_tensor(
                    osb_view(g, 16, 17).squeeze(2),
                    e4.rearrange("p (b c) -> p b c", c=CARD),
                    r1.unsqueeze(2).broadcast_to([128, 4, CARD]),
                    op=ALU.mult)
                for bti in range(4):
                    bt = 4 * g + bti
                    nc.sync.dma_start(
                        out=out_d.ap()[bt * 128:(bt + 1) * 128, :],
                        in_=osb_gs[g][:, bti * OUTW:(bti + 1) * OUTW])

            # ================= emission schedule =================
            # PE order: enc(h0) | vals+scorer(h0) with bt4-7 transposes
            # sprinkled | enc(h1) with h0 root-tail PE interleaved |
            # vals+scorer(h1) with h0 mid/task tail interleaved | tail(h1).
            encoder_half(0)
            for g in range(2):
                for lv in range(2):
                    make_ohm(g, lv)

            lg0 = ps_lg.tile([128, 128], F32, tag="lg", name="lg0")
            vals_scorer_half(
                0, lg0,
                extra_pe={1: lambda: transpose_bt(4),
                          3: lambda: transpose_bt(5),
                          5: lambda: transpose_bt(6),
                          7: lambda: transpose_bt(7)})

            # h0 root softmax chain (DVE/ACT) runs under enc(h1) on the PE
            pfin0 = softmax_chain(0, lg0, 0)
            encoder_half(1)
            p_transposes(0, pfin0, prT_sb)
            osb_store(0, pfin0, 0)
            mid_h_mms(0)
            ml0 = ps_lg.tile([128, 128], F32, tag="lg", name="ml0")
            mid_logit_mms(0, ml0)

            lg1 = ps_lg.tile([128, 128], F32, tag="lg", name="lg1")

            def h0_mid_tail():
                pf = softmax_chain(0, ml0, 1)
                p_transposes(0, pf, pmT_sb)
                osb_store(0, pf, 1)

            def h0_task():
                yl0 = ps_lg.tile([128, 16], F32, tag="lg", name="yl0")
                task_mms(0, yl0)
                y_tail(0, yl0)

            vals_scorer_half(1, lg1,
                             extra_pe={1: h0_mid_tail, 4: h0_task})

            warm_i = [0]

            def warm(n):
                ps = ps_mm.tile([128, 512], F32, tag="mm",
                                name=f"warm{warm_i[0]}")
                warm_i[0] += 1
                for _ in range(n):
                    nc.tensor.matmul(ps[:, 0:128], _mm(ident_sb),
                                     _mm(ident_sb), start=True, stop=True)

            # ---------------- h1 tail (end of kernel) ----------------
            pfin1 = softmax_chain(1, lg1, 0)
            warm(10)
            p_transposes(1, pfin1, prT_sb)
            osb_store(1, pfin1, 0)
            warm(4)
            mid_h_mms(1)
            ml1 = ps_lg.tile([128, 128], F32, tag="lg", name="ml1")
            mid_logit_mms(1, ml1)
            warm(10)
            pf = softmax_chain(1, ml1, 1)
            p_transposes(1, pf, pmT_sb)
            osb_store(1, pf, 1)
            warm(6)
            yl1 = ps_lg.tile([128, 16], F32, tag="lg", name="yl1")
            task_mms(1, yl1)
            warm(4)
            y_tail(1, yl1)

    nc.compile()
    return nc


def prep_weights(inp):
    """Host-side reformatting of (replicated) weights to device layouts."""
    mdt_np = ml_dtypes.bfloat16 if MM_DTYPE == "bf16" else np.float32
    f32 = np.float32
    W_enc = np.asarray(inp["W_enc"], f32)
    Wv = np.asarray(inp["Wv"], f32)
    Ws = np.asarray(inp["Ws"], f32)
    W1c = np.asarray(inp["W1c"], f32)
    W2c = np.asarray(inp["W2c"], f32)
    W1y = np.asarray(inp["W1y"], f32)
    W2y = np.asarray(inp["W2y"], f32)
    b1c = np.asarray(inp["b1c"], f32)
    b1y = np.asarray(inp["b1y"], f32)
    b2y = np.asarray(inp["b2y"], f32)

    # W2c block-pair layout: [s*64+h, q, s'*4+c] = W2c[2q+s', h, c] iff s==s'
    w2c_bp = np.zeros((2, 64, 4, 2, 4), f32)
    for q in range(4):
        for s in range(2):
            w2c_bp[s, :, q, s, :] = W2c[2 * q + s]  # [h, c]

    # W1c pair layout [32, 4, 2*64] + b1c ones-row -> [33, 512]
    w1c_flat = W1c.transpose(1, 0, 2).reshape(P_IN, 512)
    b1c_row = b1c.reshape(4, 2, 64).reshape(1, 512)
    w1c_aug = np.concatenate([w1c_flat, b1c_row], axis=0)

    w1y_aug = np.concatenate([W1y, b1y.reshape(1, P_HID)], axis=0)
    w2y_aug = np.concatenate([W2y, b2y.reshape(1, CARD)], axis=0)

    wmap = {
        "wenc": np.ascontiguousarray(W_enc, mdt_np),
        "wv": np.ascontiguousarray(
            Wv.transpose(1, 0, 2).reshape(D_H, N_ROOT * OV), mdt_np),
        "ws": np.ascontiguousarray(
            Ws.transpose(1, 0, 2).reshape(OV, N_ROOT * CARD), mdt_np),
        "w1c": np.ascontiguousarray(w1c_aug, mdt_np),
        "w2c": np.ascontiguousarray(w2c_bp.reshape(128, 32), mdt_np),
        "w1y": np.ascontiguousarray(w1y_aug, mdt_np),
        "w2y": np.ascontiguousarray(w2y_aug, mdt_np),
        "benc": np.ascontiguousarray(
            np.asarray(inp["b_enc"], f32).reshape(KT_H, 128).T),
        "bv": np.ascontiguousarray(
            np.asarray(inp["bv"], f32).reshape(N_ROOT, 2, 128)
            .transpose(2, 0, 1).reshape(128, 16)),
        "bsr": np.ascontiguousarray(
            np.asarray(inp["bs"], f32).reshape(1, 32), mdt_np),
        "b2cr": np.ascontiguousarray(
            np.asarray(inp["b2c"], f32).reshape(1, 32), mdt_np),
        "ident": np.ascontiguousarray(np.eye(128), mdt_np),
        "identf": np.ascontiguousarray(np.eye(128), f32),
        "iotaf": np.ascontiguousarray(
            np.tile(np.arange(CARD, dtype=f32), (128, N_ROOT))),
    }
    return wmap


def make_in_maps(inp):
    wmap = prep_weights(inp)
    x = np.ascontiguousarray(np.asarray(inp["x"], np.float32))
    lab = np.ascontiguousarray(np.asarray(inp["c"], np.int32))
    msk = np.ascontiguousarray(np.asarray(inp["intervention_index"], np.int32))
    in_maps = []
    for i in range(N_CORES):
        m = dict(wmap)
        m["x"] = x[i * BSH:(i + 1) * BSH]
        m["lab"] = lab[i * BSH:(i + 1) * BSH]
        m["msk"] = msk[i * BSH:(i + 1) * BSH]
        in_maps.append(m)
    return in_maps


_NC_CACHE = {}


def _get_nc():
    key = (MM_DTYPE, SIM_SAFE)
    if key not in _NC_CACHE:
        _NC_CACHE[key] = build_program()
    return _NC_CACHE[key]


def kernel(**inputs):
    from concourse.bass_utils import run_bass_kernel_spmd

    nc = _get_nc()
    in_maps = make_in_maps(inputs)
    res = run_bass_kernel_spmd(nc, in_maps, list(range(N_CORES)))
    outs = [np.asarray(res.results[i]["out"], np.float32).reshape(BSH, 17, CARD)
            for i in range(N_CORES)]
    return np.concatenate(outs, axis=0)



# revision 8
# speedup vs baseline: 1.4900x; 1.4900x over previous
"""Trainium2 Bass kernel for nn_C2BM_30537217474758 (gnn_message_passing).

Concept-bottleneck model:
  x_enc = lrelu(x @ W_enc + b_enc)                         [B, 1024]
  vals  = lrelu(einsum('bi,rio->bro', x_enc, Wv) + bv)     [B, 8, 256]
  p_root = softmax(einsum('bro,roc->brc', vals, Ws) + bs)  [B, 8, 4]
  p_root = intervene(p_root, c[:, :8], ii[:, :8])
  h     = lrelu(einsum('bp,nph->bnh', p_root.flat, W1c) + b1c)
  p_mid = softmax(einsum('bnh,nhc->bnc', h, W2c) + b2c); intervene
  y     = softmax(lrelu(p_mid.flat @ W1y + b1y) @ W2y + b2y)
  out   = concat([p_root, p_mid, y[:, None]], axis=1)      [B, 17, 4]

Strategy: pure data-parallel over 8 NeuronCores (batch shard 1024/core),
weights replicated.  The two large GEMMs (encoder and value-embedding,
~4.3 GFLOP each per core) run in fp8(e4m3) with DoubleRow perf mode (2x PE
throughput, fp32 PSUM accumulation); weights and x are pre-scaled on the
host (x*32, W*256) so fp8 quantization happens in the normal range, and
the scales are divided back out in the activation (lrelu is positively
homogeneous).  x is transposed and cast on the HOST, so the kernel does
zero on-chip transposition of x.

The scorer and mid/task propagators produce logits directly in TRANSPOSED
layout [32 = 8grp x 4card, batch] by using zero-padded block stationary
matrices, so softmax group sums become one tiny block-diagonal matmul and
the resulting probability tensor feeds the next propagator GEMM with no
transpose on the critical path.  Intervention one-hots/masks are
precomputed on the host in the same transposed layout.  Output staging
[batch, 68] is produced by small PE transposes off the critical path, and
the final DRAM output is [128, 8*68] per core, unsharded on the host.

Batch is processed in two 512-row halves so each half's softmax ->
propagator -> task tail (DVE/ACT latency chains) hides under the other
half's GEMMs.
"""

import os
import sys

try:
    import concourse  # noqa: F401
except ImportError:
    sys.path.insert(0, "/opt/trn_rl_repo")

import numpy as np
import ml_dtypes

import concourse.bacc as bacc
import concourse.tile as tile
from concourse import mybir

# ---------------- problem constants (hardcoded per contract) ----------------
B, D_IN, D_H = 8192, 2048, 1024
N_ROOT, N_MID, CARD, CHS = 8, 8, 4, 64
OV = CARD * CHS           # 256  value-embedding width per root
P_IN = N_ROOT * CARD      # 32
P_HID = 2 * P_IN          # 64
N_CORES = 8
BSH = B // N_CORES        # 1024 batch rows per core
KT_IN = D_IN // 128       # 16 contraction tiles for encoder
KT_H = D_H // 128         # 8 contraction tiles for Wv
OUTW = 17 * CARD          # 68 output cols per row

F32 = mybir.dt.float32
I32 = mybir.dt.int32
U8 = mybir.dt.uint8
BF16 = mybir.dt.bfloat16
FP8 = mybir.dt.float8e4
AF = mybir.ActivationFunctionType
ALU = mybir.AluOpType
AX = mybir.AxisListType
DR = mybir.MatmulPerfMode.DoubleRow

LRELU_ALPHA = 0.01
# host-side pre-scales so fp8 values land in the normal range
SX = 32.0                 # x and x_enc scale
SW = 256.0                # W_enc / Wv scale
# CoreSim does not implement Lrelu; BASS_SIM_SAFE=1 swaps in Relu so the
# rest of the program can be validated in simulation.
SIM_SAFE = os.environ.get("BASS_SIM_SAFE") == "1"
ACT_LRELU = AF.Relu if SIM_SAFE else AF.Lrelu


def build_program():
    """Emit the per-core Bass program (identical on all 8 cores)."""
    nc = bacc.Bacc("TRN2", target_bir_lowering=False, debug=False,
                   num_devices=N_CORES)

    # ------------- DRAM I/O (all host-prepped layouts) -------------
    # xt: [p, half, kt, b] = 32*x[g*512+b, kt*128+p] in fp8
    xt_d = nc.dram_tensor("xt", [128, 2 * KT_IN * 512], FP8,
                          kind="ExternalInput")
    # wenc: [p, ht, kt, c] = 256*W_enc[kt*128+p, ht*128+c]
    wenc_d = nc.dram_tensor("wenc", [128, KT_H * KT_IN * 128], FP8,
                            kind="ExternalInput")
    # wv: [p, r, kt, oc] = 256*Wv[r, kt*128+p, oc]
    wv_d = nc.dram_tensor("wv", [128, N_ROOT * KT_H * OV], FP8,
                          kind="ExternalInput")
    # ws_big: [ch, 2r+ot, 4r+c] block layout (zeros elsewhere)
    wsb_d = nc.dram_tensor("wsb", [128, 16 * 32], BF16, kind="ExternalInput")
    # w1c pair layout [32, 4, 128] + b1c ones-row -> [33, 512]
    w1c_d = nc.dram_tensor("w1c", [P_IN + 1, 4 * 128], BF16,
                           kind="ExternalInput")
    # w2c_big: [64s+h, q, 4(2q+s)+c] block layout
    w2cb_d = nc.dram_tensor("w2cb", [128, 4 * 32], BF16, kind="ExternalInput")
    w1y_d = nc.dram_tensor("w1y", [P_IN + 1, P_HID], BF16,
                           kind="ExternalInput")
    w2y_d = nc.dram_tensor("w2y", [P_HID + 1, CARD], BF16,
                           kind="ExternalInput")
    benc_d = nc.dram_tensor("benc", [128, KT_H], F32, kind="ExternalInput")
    bv_d = nc.dram_tensor("bv", [128, 16], F32, kind="ExternalInput")
    bsT_d = nc.dram_tensor("bsT", [P_IN, 1], F32, kind="ExternalInput")
    b2cT_d = nc.dram_tensor("b2cT", [P_IN, 1], F32, kind="ExternalInput")
    # transposed one-hots (bf16) and masks (u8): [4g+c | 4n+c, b]
    ohr_d = nc.dram_tensor("ohr", [P_IN, BSH], BF16, kind="ExternalInput")
    ohm_d = nc.dram_tensor("ohm", [P_IN, BSH], BF16, kind="ExternalInput")
    mr_d = nc.dram_tensor("mr", [P_IN, BSH], U8, kind="ExternalInput")
    mm_d = nc.dram_tensor("mm", [P_IN, BSH], U8, kind="ExternalInput")
    g32_d = nc.dram_tensor("g32", [P_IN, P_IN], BF16, kind="ExternalInput")
    ident_d = nc.dram_tensor("ident", [P_IN, P_IN], BF16,
                             kind="ExternalInput")
    # out: [p, bt, 68]
    out_d = nc.dram_tensor("out", [128, (BSH // 128) * OUTW], F32,
                           kind="ExternalOutput")

    with tile.TileContext(nc) as tc:
        with (
            tc.tile_pool(name="persist", bufs=1) as persist,
            tc.tile_pool(name="vals", bufs=3) as vals_pool,
            tc.tile_pool(name="tmp", bufs=2) as tmp_pool,
            tc.tile_pool(name="ps_mm", bufs=3, space="PSUM") as ps_mm,
            tc.tile_pool(name="ps_lg", bufs=1, space="PSUM") as ps_lg,
            tc.tile_pool(name="ps_sm", bufs=2, space="PSUM") as ps_sm,
        ):
            # -------- DMA order: x h0 + wenc ht0 gate the encoder ----------
            xt_sb = persist.tile([128, 2, KT_IN, 512], FP8)
            xt_r = xt_d.ap().rearrange("p (g k b) -> p g k b", g=2, b=512)
            wenc_sb = persist.tile([128, KT_H, KT_IN, 128], FP8)
            wenc_r = wenc_d.ap().rearrange("p (h k c) -> p h k c",
                                           h=KT_H, c=128)
            nc.gpsimd.dma_start(out=xt_sb[:, 0], in_=xt_r[:, 0])
            for ht in range(2):
                nc.sync.dma_start(out=wenc_sb[:, ht], in_=wenc_r[:, ht])
            nc.gpsimd.dma_start(out=xt_sb[:, 1], in_=xt_r[:, 1])

            # small tensors (needed from the root-softmax tail onward)
            wsb_sb = persist.tile([128, 16, 32], BF16)
            nc.sync.dma_start(out=wsb_sb,
                              in_=wsb_d.ap().rearrange("p (q c) -> p q c",
                                                       c=32))
            w1c_sb = persist.tile([P_IN + 1, 4, 128], BF16)
            nc.sync.dma_start(out=w1c_sb,
                              in_=w1c_d.ap().rearrange("p (q m) -> p q m",
                                                       m=128))
            w2cb_sb = persist.tile([128, 4, 32], BF16)
            nc.sync.dma_start(out=w2cb_sb,
                              in_=w2cb_d.ap().rearrange("p (q c) -> p q c",
                                                        c=32))
            w1y_sb = persist.tile([P_IN + 1, P_HID], BF16)
            nc.sync.dma_start(out=w1y_sb, in_=w1y_d.ap())
            w2y_sb = persist.tile([P_HID + 1, CARD], BF16)
            nc.sync.dma_start(out=w2y_sb, in_=w2y_d.ap())
            benc_sb = persist.tile([128, KT_H], F32)
            nc.sync.dma_start(out=benc_sb, in_=benc_d.ap())
            bv_sb = persist.tile([128, 16], F32)
            nc.sync.dma_start(out=bv_sb, in_=bv_d.ap())
            bsT_sb = persist.tile([P_IN, 1], F32)
            nc.sync.dma_start(out=bsT_sb, in_=bsT_d.ap())
            b2cT_sb = persist.tile([P_IN, 1], F32)
            nc.sync.dma_start(out=b2cT_sb, in_=b2cT_d.ap())
            ohr_sb = persist.tile([P_IN, BSH], BF16)
            nc.sync.dma_start(out=ohr_sb, in_=ohr_d.ap())
            ohm_sb = persist.tile([P_IN, BSH], BF16)
            nc.sync.dma_start(out=ohm_sb, in_=ohm_d.ap())
            mr_sb = persist.tile([P_IN, BSH], U8)
            nc.sync.dma_start(out=mr_sb, in_=mr_d.ap())
            mm_sb = persist.tile([P_IN, BSH], U8)
            nc.sync.dma_start(out=mm_sb, in_=mm_d.ap())
            g32_sb = persist.tile([P_IN, P_IN], BF16)
            nc.sync.dma_start(out=g32_sb, in_=g32_d.ap())
            ident_sb = persist.tile([P_IN, P_IN], BF16)
            nc.sync.dma_start(out=ident_sb, in_=ident_d.ap())

            for ht in range(2, KT_H):
                nc.sync.dma_start(out=wenc_sb[:, ht], in_=wenc_r[:, ht])
            wv_sb = persist.tile([128, N_ROOT, KT_H, OV], FP8)
            wv_r = wv_d.ap().rearrange("p (r k o) -> p r k o",
                                       r=N_ROOT, o=OV)
            for r in range(N_ROOT):
                nc.sync.dma_start(out=wv_sb[:, r], in_=wv_r[:, r])

            # ---------------- persistent activations ----------------
            xenc_sb = persist.tile([128, KT_H, BSH], FP8)   # 32*x_encT
            prT_sb = persist.tile([P_IN + 1, BSH], BF16)    # row 32 = ones
            nc.vector.memset(prT_sb[P_IN:P_IN + 1, :], 1.0)
            pmT_sb = persist.tile([P_IN + 1, BSH], BF16)
            nc.vector.memset(pmT_sb[P_IN:P_IN + 1, :], 1.0)
            hyT_sb = persist.tile([P_HID + 1, BSH], BF16)   # row 64 = ones
            nc.vector.memset(hyT_sb[P_HID:P_HID + 1, :], 1.0)
            hT_sb = persist.tile([128, 4, BSH], BF16)  # [2 mids x 64h, b]
            osb = persist.tile([128, BSH // 128, OUTW], F32)

            # ---------------- encoder GEMM -> x_encT (fp8) ----------------
            def encoder_half(g):
                for ht in range(KT_H):
                    ps = ps_mm.tile([128, 512], F32, tag="mm")
                    for c in range(2):
                        for j in range(KT_IN // 2):
                            nc.tensor.matmul(
                                ps[:, c * 256:(c + 1) * 256],
                                wenc_sb[:, ht, 2 * j:2 * j + 2, :],
                                xt_sb[:, g, 2 * j:2 * j + 2,
                                      c * 256:(c + 1) * 256],
                                start=(j == 0), stop=(j == KT_IN // 2 - 1),
                                perf_mode=DR, skip_group_check=(c == 1))
                    nc.scalar.activation(
                        xenc_sb[:, ht, g * 512:(g + 1) * 512], ps,
                        ACT_LRELU, bias=benc_sb[:, ht:ht + 1],
                        scale=float(SX / (SX * SW)), alpha=LRELU_ALPHA)

            # ------------- per-root value GEMM + scorer (one half) ----------
            def vals_scorer_half(g, lg):
                """Value embeddings + scorer; logitsT into lg [32, 512]."""
                for r in range(N_ROOT):
                    vt = vals_pool.tile([128, 2, 512], BF16, tag="vals")
                    for ot in range(2):
                        ps = ps_mm.tile([128, 512], F32, tag="mm")
                        for c in range(2):
                            for j in range(KT_H // 2):
                                nc.tensor.matmul(
                                    ps[:, c * 256:(c + 1) * 256],
                                    wv_sb[:, r, 2 * j:2 * j + 2,
                                          ot * 128:(ot + 1) * 128],
                                    xenc_sb[:, 2 * j:2 * j + 2,
                                            g * 512 + c * 256:
                                            g * 512 + (c + 1) * 256],
                                    start=(j == 0), stop=(j == KT_H // 2 - 1),
                                    perf_mode=DR, skip_group_check=(c == 1))
                        nc.scalar.activation(
                            vt[:, ot, :], ps, ACT_LRELU,
                            bias=bv_sb[:, 2 * r + ot:2 * r + ot + 1],
                            scale=float(1.0 / (SX * SW)), alpha=LRELU_ALPHA)
                    for ot in range(2):
                        nc.tensor.matmul(
                            lg, wsb_sb[:, 2 * r + ot, :], vt[:, ot, :],
                            start=(r == 0 and ot == 0),
                            stop=(r == N_ROOT - 1 and ot == 1))

            # ------------- transposed softmax + intervention tail ----------
            def softmax_chain(g, lg, bias, oh, m, pT):
                """softmax+intervene on logitsT lg [32,512](PSUM);
                probs -> pT[0:32, g*512:(g+1)*512] (bf16)."""
                cols = slice(g * 512, (g + 1) * 512)
                e = tmp_pool.tile([P_IN, 512], BF16, tag="e", bufs=3)
                nc.scalar.activation(e, lg, AF.Exp, bias=bias)
                sm = ps_sm.tile([P_IN, 512], F32, tag="sums", bufs=1)
                nc.tensor.matmul(sm, g32_sb, e, start=True, stop=True)
                rcp = tmp_pool.tile([P_IN, 512], F32, tag="rcp", bufs=2)
                nc.vector.reciprocal(rcp, sm)
                nc.vector.tensor_tensor(pT[0:P_IN, cols], e, rcp, op=ALU.mult)
                nc.vector.copy_predicated(pT[0:P_IN, cols], m[:, cols],
                                          oh[:, cols])

            def mid_h_mms(g, q):
                ps = ps_mm.tile([128, 512], F32, tag="mm")
                nc.tensor.matmul(
                    ps, w1c_sb[:, q, :],
                    prT_sb[:, g * 512:(g + 1) * 512], start=True, stop=True)
                dst = hT_sb[:, q, g * 512:(g + 1) * 512]
                if SIM_SAFE:
                    nc.vector.tensor_scalar(dst, ps, 0.0, None, op0=ALU.max)
                else:
                    t = tmp_pool.tile([128, 512], BF16, tag="lr", bufs=2)
                    nc.vector.tensor_scalar(t, ps, LRELU_ALPHA, None,
                                            op0=ALU.mult)
                    nc.vector.tensor_tensor(dst, ps, t, op=ALU.max)

            def mid_logit_mms(g, ml):
                for q in range(4):
                    nc.tensor.matmul(
                        ml, w2cb_sb[:, q, :],
                        hT_sb[:, q, g * 512:(g + 1) * 512],
                        start=(q == 0), stop=(q == 3))

            def task_mms(g, yl):
                ps = ps_mm.tile([P_HID, 512], F32, tag="mm")
                nc.tensor.matmul(
                    ps, w1y_sb, pmT_sb[:, g * 512:(g + 1) * 512],
                    start=True, stop=True)
                dst = hyT_sb[0:P_HID, g * 512:(g + 1) * 512]
                if SIM_SAFE:
                    nc.vector.tensor_scalar(dst, ps, 0.0, None, op0=ALU.max)
                else:
                    t = tmp_pool.tile([P_HID, 512], BF16, tag="lry", bufs=2)
                    nc.vector.tensor_scalar(t, ps, LRELU_ALPHA, None,
                                            op0=ALU.mult)
                    nc.vector.tensor_tensor(dst, ps, t, op=ALU.max)
                for bti in range(4):
                    bt = 4 * g + bti
                    nc.tensor.matmul(
                        yl[:, bti * 4:(bti + 1) * 4],
                        hyT_sb[:, bt * 128:(bt + 1) * 128], w2y_sb,
                        start=True, stop=True, skip_group_check=True)

            def y_tail(g, yl):
                e4 = tmp_pool.tile([128, 16], F32, tag="e4")
                nc.scalar.activation(e4, yl, AF.Exp)
                s1 = tmp_pool.tile([128, 4], F32, tag="s1")
                nc.vector.reduce_sum(
                    s1, e4.rearrange("p (b c) -> p b c", c=CARD), axis=AX.X)
                r1 = tmp_pool.tile([128, 4], F32, tag="r1")
                nc.vector.reciprocal(r1, s1)
                nc.vector.tensor_tensor(
                    osb[:, 4 * g:4 * g + 4, 16 * CARD:17 * CARD],
                    e4.rearrange("p (b c) -> p b c", c=CARD),
                    r1.unsqueeze(2).broadcast_to([128, 4, CARD]),
                    op=ALU.mult)

            def osb_transposes(g, pT, lv):
                """pT[0:32, half g] -> osb[:, bt, lv*32:(lv+1)*32]."""
                for bti in range(4):
                    bt = 4 * g + bti
                    trp = ps_sm.tile([128, P_IN], BF16, tag="ptr", bufs=2)
                    nc.tensor.transpose(
                        trp, pT[0:P_IN, bt * 128:(bt + 1) * 128], ident_sb)
                    nc.vector.tensor_copy(
                        osb[:, bt, lv * P_IN:(lv + 1) * P_IN], trp)

            def out_dma(g):
                o_r = out_d.ap().rearrange("p (t k) -> p t k", k=OUTW)
                nc.sync.dma_start(out=o_r[:, 4 * g:4 * g + 4],
                                  in_=osb[:, 4 * g:4 * g + 4])

            warm_i = [0]

            def warm(n):
                ps = ps_sm.tile([P_IN, 32], F32, tag="ptr",
                                name=f"warm{warm_i[0]}")
                warm_i[0] += 1
                for _ in range(n):
                    nc.tensor.matmul(ps, ident_sb, ident_sb,
                                     start=True, stop=True)

            # ================= emission schedule =================
            # PE order: enc(h0) | vals+scorer(h0) | sums r-h0 | enc(h1) |
            # mid-h/mid-logit/sums m-h0 | osb-transposes r-h0 |
            # vals+scorer(h1) | sums r-h1 | task(h0)+y | transposes m-h0 |
            # tail(h1) with warm filler.
            encoder_half(0)
            lg0 = ps_lg.tile([P_IN, 512], F32, tag="lgr", name="lg0")
            vals_scorer_half(0, lg0)
            softmax_chain(0, lg0, bsT_sb, ohr_sb, mr_sb, prT_sb)
            encoder_half(1)
            for q in range(4):
                mid_h_mms(0, q)
            ml0 = ps_lg.tile([P_IN, 512], F32, tag="lgm", name="ml0")
            mid_logit_mms(0, ml0)
            softmax_chain(0, ml0, b2cT_sb, ohm_sb, mm_sb, pmT_sb)
            osb_transposes(0, prT_sb, 0)

            lg1 = ps_lg.tile([P_IN, 512], F32, tag="lgr", name="lg1")
            vals_scorer_half(1, lg1)

            # h0 tail: task + y + output staging (hides under nothing on PE,
            # but all its PE ops are tiny; DVE/ACT run under vals h1)
            yl0 = ps_sm.tile([128, 16], F32, tag="ptr", name="yl0")
            task_mms(0, yl0)
            y_tail(0, yl0)
            osb_transposes(0, pmT_sb, 1)
            out_dma(0)

            # ---------------- h1 tail (end of kernel) ----------------
            softmax_chain(1, lg1, bsT_sb, ohr_sb, mr_sb, prT_sb)
            warm(8)
            for q in range(4):
                mid_h_mms(1, q)
            ml1 = ps_lg.tile([P_IN, 512], F32, tag="lgm", name="ml1")
            mid_logit_mms(1, ml1)
            softmax_chain(1, ml1, b2cT_sb, ohm_sb, mm_sb, pmT_sb)
            osb_transposes(1, prT_sb, 0)
            warm(8)
            yl1 = ps_sm.tile([128, 16], F32, tag="ptr", name="yl1")
            task_mms(1, yl1)
            warm(4)
            y_tail(1, yl1)
            osb_transposes(1, pmT_sb, 1)
            out_dma(1)

    nc.compile()
    return nc


def prep_weights(inp):
    """Host-side reformatting of (replicated) weights to device layouts."""
    f32 = np.float32
    fp8 = ml_dtypes.float8_e4m3

    def to_fp8(a):
        return np.clip(a, -240.0, 240.0).astype(fp8)

    W_enc = np.asarray(inp["W_enc"], f32)          # [2048, 1024]
    Wv = np.asarray(inp["Wv"], f32)                # [8, 1024, 256]
    Ws = np.asarray(inp["Ws"], f32)                # [8, 256, 4]
    W1c = np.asarray(inp["W1c"], f32)              # [8, 32, 64]
    W2c = np.asarray(inp["W2c"], f32)              # [8, 64, 4]
    W1y = np.asarray(inp["W1y"], f32)              # [32, 64]
    W2y = np.asarray(inp["W2y"], f32)              # [64, 4]
    b1c = np.asarray(inp["b1c"], f32)
    b1y = np.asarray(inp["b1y"], f32)
    b2y = np.asarray(inp["b2y"], f32)

    # wenc [p, ht, kt, c]
    wenc = (SW * W_enc).reshape(KT_IN, 128, KT_H, 128).transpose(1, 2, 0, 3)
    # wv [p, r, kt, oc]
    wv = (SW * Wv).reshape(N_ROOT, KT_H, 128, OV).transpose(2, 0, 1, 3)
    # ws_big [ch, 2r+ot, 4r+c]
    wsb = np.zeros((128, 16, 32), f32)
    for r in range(N_ROOT):
        for ot in range(2):
            wsb[:, 2 * r + ot, 4 * r:4 * r + 4] = \
                Ws[r, ot * 128:(ot + 1) * 128, :]
    # w1c pair layout [32, 4, 128] + b1c ones-row -> [33, 512]
    w1c_flat = W1c.transpose(1, 0, 2).reshape(P_IN, 512)
    w1c_aug = np.concatenate([w1c_flat, b1c.reshape(1, 512)], axis=0)
    # w2c_big [64s+h, q, 4(2q+s)+c]
    w2cb = np.zeros((128, 4, 32), f32)
    for q in range(4):
        for s in range(2):
            w2cb[64 * s:64 * s + 64, q, 4 * (2 * q + s):4 * (2 * q + s) + 4] \
                = W2c[2 * q + s]
    w1y_aug = np.concatenate([W1y, b1y.reshape(1, P_HID)], axis=0)
    w2y_aug = np.concatenate([W2y, b2y.reshape(1, CARD)], axis=0)
    # block-diagonal group-sum matrix
    g32 = np.kron(np.eye(8, dtype=f32), np.ones((4, 4), f32))

    bf16 = ml_dtypes.bfloat16
    wmap = {
        "wenc": np.ascontiguousarray(to_fp8(wenc).reshape(128, -1)),
        "wv": np.ascontiguousarray(to_fp8(wv).reshape(128, -1)),
        "wsb": np.ascontiguousarray(wsb.reshape(128, -1), dtype=bf16),
        "w1c": np.ascontiguousarray(w1c_aug, dtype=bf16),
        "w2cb": np.ascontiguousarray(w2cb.reshape(128, -1), dtype=bf16),
        "w1y": np.ascontiguousarray(w1y_aug, dtype=bf16),
        "w2y": np.ascontiguousarray(w2y_aug, dtype=bf16),
        "benc": np.ascontiguousarray(
            (SX * np.asarray(inp["b_enc"], f32)).reshape(KT_H, 128).T),
        "bv": np.ascontiguousarray(
            np.asarray(inp["bv"], f32).reshape(N_ROOT, 2, 128)
            .transpose(2, 0, 1).reshape(128, 16)),
        "bsT": np.ascontiguousarray(
            np.asarray(inp["bs"], f32).reshape(P_IN, 1)),
        "b2cT": np.ascontiguousarray(
            np.asarray(inp["b2c"], f32).reshape(P_IN, 1)),
        "g32": np.ascontiguousarray(g32, dtype=bf16),
        "ident": np.ascontiguousarray(np.eye(P_IN), dtype=bf16),
    }
    return wmap


def make_in_maps(inp):
    f32 = np.float32
    fp8 = ml_dtypes.float8_e4m3
    bf16 = ml_dtypes.bfloat16
    wmap = prep_weights(inp)
    x = np.asarray(inp["x"], f32)
    lab = np.asarray(inp["c"], np.int32)
    msk = np.asarray(inp["intervention_index"], np.int32)

    # transposed one-hot / mask tensors, [4grp+c, b] per core
    iot = np.arange(CARD, dtype=np.int32)
    in_maps = []
    for i in range(N_CORES):
        m = dict(wmap)
        xc = x[i * BSH:(i + 1) * BSH]                     # [1024, 2048]
        xt = np.clip(SX * xc, -240.0, 240.0).astype(fp8)
        xt = xt.reshape(2, 512, KT_IN, 128).transpose(3, 0, 2, 1)
        m["xt"] = np.ascontiguousarray(xt.reshape(128, -1))
        lc = lab[i * BSH:(i + 1) * BSH]                   # [1024, 17]
        mc = msk[i * BSH:(i + 1) * BSH]
        # ohr[4r+c, b] = (lab[b, r] == c)
        ohr = (lc[:, :8, None] == iot).transpose(1, 2, 0).reshape(P_IN, BSH)
        ohm = (lc[:, 8:16, None] == iot).transpose(1, 2, 0).reshape(P_IN, BSH)
        mrr = np.repeat(mc[:, :8].T, CARD, axis=0)        # [32, 1024]
        mmm = np.repeat(mc[:, 8:16].T, CARD, axis=0)
        m["ohr"] = np.ascontiguousarray(ohr.astype(bf16))
        m["ohm"] = np.ascontiguousarray(ohm.astype(bf16))
        m["mr"] = np.ascontiguousarray(mrr.astype(np.uint8))
        m["mm"] = np.ascontiguousarray(mmm.astype(np.uint8))
        in_maps.append(m)
    return in_maps


def unshard_out(res_out):
    """[128, 8*68] per-core DRAM layout -> [BSH, 17, 4]."""
    a = np.asarray(res_out, np.float32).reshape(128, BSH // 128, 17, CARD)
    return np.ascontiguousarray(a.transpose(1, 0, 2, 3)).reshape(
        BSH, 17, CARD)


_NC_CACHE = {}


def _get_nc():
    key = SIM_SAFE
    if key not in _NC_CACHE:
        _NC_CACHE[key] = build_program()
    return _NC_CACHE[key]


def kernel(**inputs):
    from concourse.bass_utils import run_bass_kernel_spmd

    nc = _get_nc()
    in_maps = make_in_maps(inputs)
    res = run_bass_kernel_spmd(nc, in_maps, list(range(N_CORES)))
    outs = [unshard_out(res.results[i]["out"]) for i in range(N_CORES)]
    return np.concatenate(outs, axis=0)


# revision 18
# speedup vs baseline: 1.6508x; 1.1079x over previous
"""Trainium2 Bass kernel for nn_C2BM_30537217474758 (gnn_message_passing).

Concept-bottleneck model:
  x_enc = lrelu(x @ W_enc + b_enc)                         [B, 1024]
  vals  = lrelu(einsum('bi,rio->bro', x_enc, Wv) + bv)     [B, 8, 256]
  p_root = softmax(einsum('bro,roc->brc', vals, Ws) + bs)  [B, 8, 4]
  p_root = intervene(p_root, c[:, :8], ii[:, :8])
  h     = lrelu(einsum('bp,nph->bnh', p_root.flat, W1c) + b1c)
  p_mid = softmax(einsum('bnh,nhc->bnc', h, W2c) + b2c); intervene
  y     = softmax(lrelu(p_mid.flat @ W1y + b1y) @ W2y + b2y)
  out   = concat([p_root, p_mid, y[:, None]], axis=1)      [B, 17, 4]

Strategy: pure data-parallel over 8 NeuronCores (batch shard 1024/core),
weights replicated.  The two large GEMMs (encoder and value-embedding,
~4.3 GFLOP each per core) run in fp8(e4m3) with DoubleRow perf mode (2x PE
throughput, fp32 PSUM accumulation); weights and x are pre-scaled on the
host (x*32, W*256) so fp8 quantization happens in the normal range, and
the scales are divided back out in the activation (lrelu is positively
homogeneous).  x is transposed and cast on the HOST, so the kernel does
zero on-chip transposition of x.

The scorer and mid/task propagators produce logits directly in TRANSPOSED
layout [32 = 8grp x 4card, batch] by using zero-padded block stationary
matrices, so softmax group sums become one tiny block-diagonal matmul and
the resulting probability tensor feeds the next propagator GEMM with no
transpose on the critical path.  Intervention one-hots/masks are
precomputed on the host in the same transposed layout.  Output staging
[batch, 68] is produced by small PE transposes off the critical path, and
the final DRAM output is [128, 8*68] per core, unsharded on the host.

Batch is processed in two 512-row halves so each half's softmax ->
propagator -> task tail (DVE/ACT latency chains) hides under the other
half's GEMMs.
"""

import os
import sys

try:
    import concourse  # noqa: F401
except ImportError:
    sys.path.insert(0, "/opt/trn_rl_repo")

import numpy as np
import ml_dtypes

import concourse.bacc as bacc
import concourse.tile as tile
from concourse import mybir

# ---------------- problem constants (hardcoded per contract) ----------------
B, D_IN, D_H = 8192, 2048, 1024
N_ROOT, N_MID, CARD, CHS = 8, 8, 4, 64
OV = CARD * CHS           # 256  value-embedding width per root
P_IN = N_ROOT * CARD      # 32
P_HID = 2 * P_IN          # 64
N_CORES = 8
BSH = B // N_CORES        # 1024 batch rows per core
KT_IN = D_IN // 128       # 16 contraction tiles for encoder
KT_H = D_H // 128         # 8 contraction tiles for Wv
OUTW = 17 * CARD          # 68 output cols per row

F32 = mybir.dt.float32
I32 = mybir.dt.int32
U8 = mybir.dt.uint8
BF16 = mybir.dt.bfloat16
FP8 = mybir.dt.float8e4
AF = mybir.ActivationFunctionType
ALU = mybir.AluOpType
AX = mybir.AxisListType
DR = mybir.MatmulPerfMode.DoubleRow

LRELU_ALPHA = 0.01
# host-side pre-scales so fp8 values land in the normal range
SX = 32.0                 # x and x_enc scale
SW = 256.0                # W_enc / Wv scale
# CoreSim does not implement Lrelu; BASS_SIM_SAFE=1 swaps in Relu so the
# rest of the program can be validated in simulation.
SIM_SAFE = os.environ.get("BASS_SIM_SAFE") == "1"
ACT_LRELU = AF.Relu if SIM_SAFE else AF.Lrelu


def build_program():
    """Emit the per-core Bass program (identical on all 8 cores)."""
    nc = bacc.Bacc("TRN2", target_bir_lowering=False, debug=False,
                   num_devices=N_CORES)

    # ------------- DRAM I/O (all host-prepped layouts) -------------
    # xt: [p, half, kt, b] = 32*x[g*512+b, kt*128+p] in fp8
    xt_d = nc.dram_tensor("xt", [128, 2 * KT_IN * 512], FP8,
                          kind="ExternalInput")
    # wenc: [p, ht, kt, c] = 256*W_enc[kt*128+p, ht*128+c]
    wenc_d = nc.dram_tensor("wenc", [128, KT_H * KT_IN * 128], FP8,
                            kind="ExternalInput")
    # wv: [p, r, kt, oc] = 256*Wv[r, kt*128+p, oc]
    wv_d = nc.dram_tensor("wv", [128, N_ROOT * KT_H * OV], FP8,
                          kind="ExternalInput")
    # packed fp32 constants: benc(0:8) | bv(8:24) | bsT col 24 | b2cT col 25
    cstf_d = nc.dram_tensor("cstf", [128, 26], F32, kind="ExternalInput")
    # packed bf16 constants: ws_big [ch, 2r+ot, 4r+c] (cols 0:512) |
    # w2c_big [64s+h, q, 4(2q+s)+c] (512:640) | w1c+b1c [33, 4, 128]
    # (640:1152) | w1y+b1y [33, 64] (1152:1216) | w2y+b2y [65, 4]
    # (1216:1220) | ident32 (1220:1252) | g32 (1252:1284)
    cstb_d = nc.dram_tensor("cstb", [128, 1284], BF16, kind="ExternalInput")
    # transposed one-hots (bf16) and masks (u8): [4g+c | 4n+c, b];
    # cols 0:1024 = root level, 1024:2048 = mid level
    ohb_d = nc.dram_tensor("ohb", [P_IN, 2 * BSH], BF16, kind="ExternalInput")
    mb_d = nc.dram_tensor("mb", [P_IN, 2 * BSH], U8, kind="ExternalInput")
    # out: [p, bt, 68]
    out_d = nc.dram_tensor("out", [128, (BSH // 128) * OUTW], F32,
                           kind="ExternalOutput")

    with tile.TileContext(nc) as tc:
        with (
            tc.tile_pool(name="persist", bufs=1) as persist,
            tc.tile_pool(name="vals", bufs=3) as vals_pool,
            tc.tile_pool(name="tmp", bufs=2) as tmp_pool,
            tc.tile_pool(name="ps_mm", bufs=4, space="PSUM") as ps_mm,
            tc.tile_pool(name="ps_lg", bufs=1, space="PSUM") as ps_lg,
            tc.tile_pool(name="ps_sm", bufs=2, space="PSUM") as ps_sm,
        ):
            # -------- DMA order: x h0 + wenc ht0 gate the encoder ----------
            # SWDGE ring: xt halves then wv (needed from ~t+20us).
            # SP ring: wenc ht0, fp32 consts (gate the first act), rest of
            # wenc, packed bf16 consts, one-hots/masks.
            xt_sb = persist.tile([128, 2, KT_IN, 512], FP8)
            xt_r = xt_d.ap().rearrange("p (g k b) -> p g k b", g=2, b=512)
            wenc_sb = persist.tile([128, KT_H, KT_IN, 128], FP8)
            wenc_r = wenc_d.ap().rearrange("p (h k c) -> p h k c",
                                           h=KT_H, c=128)
            nc.gpsimd.dma_start(out=xt_sb[:, 0], in_=xt_r[:, 0])
            nc.sync.dma_start(out=wenc_sb[:, 0], in_=wenc_r[:, 0])
            cstf_sb = persist.tile([128, 26], F32)
            nc.sync.dma_start(out=cstf_sb, in_=cstf_d.ap())
            benc_sb = cstf_sb[:, 0:8]
            bv_sb = cstf_sb[:, 8:24]
            bsT_sb = cstf_sb[0:P_IN, 24:25]
            b2cT_sb = cstf_sb[0:P_IN, 25:26]
            nc.gpsimd.dma_start(out=xt_sb[:, 1], in_=xt_r[:, 1])
            wv_sb = persist.tile([128, N_ROOT, KT_H, OV], FP8)
            wv_r = wv_d.ap().rearrange("p (r k o) -> p r k o",
                                       r=N_ROOT, o=OV)
            for r in range(N_ROOT):
                nc.gpsimd.dma_start(out=wv_sb[:, r], in_=wv_r[:, r])
            for ht in range(1, KT_H):
                nc.sync.dma_start(out=wenc_sb[:, ht], in_=wenc_r[:, ht])

            # packed bf16 constants (one DMA): wsb | w2cb | w1c | w1y | w2y
            # | ident | g32
            cstb_sb = persist.tile([128, 1284], BF16)
            nc.sync.dma_start(out=cstb_sb, in_=cstb_d.ap())
            wsb_sb = cstb_sb[:, 0:512].rearrange("p (q c) -> p q c", c=32)
            w2cb_sb = cstb_sb[:, 512:640].rearrange("p (q c) -> p q c", c=32)
            w1c_sb = cstb_sb[0:P_IN + 1, 640:1152].rearrange(
                "p (q m) -> p q m", m=128)
            w1y_sb = cstb_sb[0:P_IN + 1, 1152:1216]
            w2y_sb = cstb_sb[0:P_HID + 1, 1216:1220]
            ident_sb = cstb_sb[0:P_IN, 1220:1252]
            g32_sb = cstb_sb[0:P_IN, 1252:1284]

            ohb_sb = persist.tile([P_IN, 2 * BSH], BF16)
            nc.sync.dma_start(out=ohb_sb, in_=ohb_d.ap())
            mb_sb = persist.tile([P_IN, 2 * BSH], U8)
            nc.sync.dma_start(out=mb_sb, in_=mb_d.ap())

            # ---------------- persistent activations ----------------
            xenc_sb = persist.tile([128, KT_H, BSH], FP8)   # 32*x_encT
            prT_sb = persist.tile([P_IN + 1, BSH], BF16)    # row 32 = ones
            nc.vector.memset(prT_sb[P_IN:P_IN + 1, :], 1.0)
            pmT_sb = persist.tile([P_IN + 1, BSH], BF16)
            nc.vector.memset(pmT_sb[P_IN:P_IN + 1, :], 1.0)
            hyT_sb = persist.tile([P_HID + 1, BSH], BF16)   # row 64 = ones
            nc.vector.memset(hyT_sb[P_HID:P_HID + 1, :], 1.0)
            hT_sb = persist.tile([128, 4, BSH], BF16)  # [2 mids x 64h, b]
            osb = persist.tile([128, BSH // 128, OUTW], F32)

            # ---------------- encoder GEMM -> x_encT (fp8) ----------------
            def encoder_half(g):
                for ht in range(KT_H):
                    ps = ps_mm.tile([128, 512], F32, tag="mm")
                    for c in range(2):
                        for j in range(KT_IN // 2):
                            nc.tensor.matmul(
                                ps[:, c * 256:(c + 1) * 256],
                                wenc_sb[:, ht, 2 * j:2 * j + 2, :],
                                xt_sb[:, g, 2 * j:2 * j + 2,
                                      c * 256:(c + 1) * 256],
                                start=(j == 0), stop=(j == KT_IN // 2 - 1),
                                perf_mode=DR, skip_group_check=(c == 1))
                    nc.scalar.activation(
                        xenc_sb[:, ht, g * 512:(g + 1) * 512], ps,
                        ACT_LRELU, bias=benc_sb[:, ht:ht + 1],
                        scale=float(SX / (SX * SW)), alpha=LRELU_ALPHA)

            # ------------- per-root value GEMM + scorer (one half) ----------
            def vals_scorer_half(g, lg):
                """Value embeddings + scorer; logitsT into lg [32, 512].
                The scorer for root r is emitted after root r+1's value
                GEMMs so the PE never stalls waiting on the vals activation
                (a stall there resets the PE pstate ramp)."""
                vts = {}

                def scorer(r):
                    for ot in range(2):
                        nc.tensor.matmul(
                            lg, wsb_sb[:, 2 * r + ot, :], vts[r][:, ot, :],
                            start=(r == 0 and ot == 0),
                            stop=(r == N_ROOT - 1 and ot == 1))

                for r in range(N_ROOT):
                    vt = vals_pool.tile([128, 2, 512], BF16, tag="vals")
                    vts[r] = vt
                    for ot in range(2):
                        ps = ps_mm.tile([128, 512], F32, tag="mm")
                        for c in range(2):
                            for j in range(KT_H // 2):
                                nc.tensor.matmul(
                                    ps[:, c * 256:(c + 1) * 256],
                                    wv_sb[:, r, 2 * j:2 * j + 2,
                                          ot * 128:(ot + 1) * 128],
                                    xenc_sb[:, 2 * j:2 * j + 2,
                                            g * 512 + c * 256:
                                            g * 512 + (c + 1) * 256],
                                    start=(j == 0), stop=(j == KT_H // 2 - 1),
                                    perf_mode=DR, skip_group_check=(c == 1))
                        nc.scalar.activation(
                            vt[:, ot, :], ps, ACT_LRELU,
                            bias=bv_sb[:, 2 * r + ot:2 * r + ot + 1],
                            scale=float(1.0 / (SX * SW)), alpha=LRELU_ALPHA)
                    if r >= 1:
                        scorer(r - 1)
                scorer(N_ROOT - 1)

            # ------------- transposed softmax + intervention tail ----------
            def softmax_chain(g, lg, bias, lv, pT):
                """softmax+intervene on logitsT lg [32,512](PSUM);
                probs -> pT[0:32, g*512:(g+1)*512] (bf16)."""
                cols = slice(g * 512, (g + 1) * 512)
                pcols = slice(lv * BSH + g * 512, lv * BSH + (g + 1) * 512)
                e = tmp_pool.tile([P_IN, 512], BF16, tag="e", bufs=3)
                nc.scalar.activation(e, lg, AF.Exp, bias=bias)
                sm = ps_sm.tile([P_IN, 512], F32, tag="sums", bufs=1)
                nc.tensor.matmul(sm, g32_sb, e, start=True, stop=True)
                rcp = tmp_pool.tile([P_IN, 512], F32, tag="rcp", bufs=2)
                nc.vector.reciprocal_approx_fast(rcp, sm)
                nc.vector.tensor_tensor(pT[0:P_IN, cols], e, rcp, op=ALU.mult)
                nc.vector.copy_predicated(pT[0:P_IN, cols], mb_sb[:, pcols],
                                          ohb_sb[:, pcols])

            def mid_h_mms(g, q):
                ps = ps_mm.tile([128, 512], F32, tag="mm")
                nc.tensor.matmul(
                    ps, w1c_sb[:, q, :],
                    prT_sb[:, g * 512:(g + 1) * 512], start=True, stop=True)
                dst = hT_sb[:, q, g * 512:(g + 1) * 512]
                if SIM_SAFE:
                    nc.vector.tensor_scalar(dst, ps, 0.0, None, op0=ALU.max)
                else:
                    t = tmp_pool.tile([128, 512], BF16, tag="lr", bufs=2)
                    nc.vector.tensor_scalar(t, ps, LRELU_ALPHA, None,
                                            op0=ALU.mult)
                    nc.vector.tensor_tensor(dst, ps, t, op=ALU.max)

            def mid_logit_mms(g, ml):
                for q in range(4):
                    nc.tensor.matmul(
                        ml, w2cb_sb[:, q, :],
                        hT_sb[:, q, g * 512:(g + 1) * 512],
                        start=(q == 0), stop=(q == 3))

            def task_mms(g, yl):
                ps = ps_mm.tile([P_HID, 512], F32, tag="mm")
                nc.tensor.matmul(
                    ps, w1y_sb, pmT_sb[:, g * 512:(g + 1) * 512],
                    start=True, stop=True)
                dst = hyT_sb[0:P_HID, g * 512:(g + 1) * 512]
                if SIM_SAFE:
                    nc.vector.tensor_scalar(dst, ps, 0.0, None, op0=ALU.max)
                else:
                    t = tmp_pool.tile([P_HID, 512], BF16, tag="lry", bufs=2)
                    nc.vector.tensor_scalar(t, ps, LRELU_ALPHA, None,
                                            op0=ALU.mult)
                    nc.vector.tensor_tensor(dst, ps, t, op=ALU.max)
                for bti in range(4):
                    bt = 4 * g + bti
                    nc.tensor.matmul(
                        yl[:, bti * 4:(bti + 1) * 4],
                        hyT_sb[:, bt * 128:(bt + 1) * 128], w2y_sb,
                        start=True, stop=True, skip_group_check=True)

            def y_tail(g, yl):
                e4 = tmp_pool.tile([128, 16], F32, tag="e4")
                nc.scalar.activation(e4, yl, AF.Exp)
                s1 = tmp_pool.tile([128, 4], F32, tag="s1")
                nc.vector.reduce_sum(
                    s1, e4.rearrange("p (b c) -> p b c", c=CARD), axis=AX.X)
                r1 = tmp_pool.tile([128, 4], F32, tag="r1")
                nc.vector.reciprocal(r1, s1)
                nc.vector.tensor_tensor(
                    osb[:, 4 * g:4 * g + 4, 16 * CARD:17 * CARD],
                    e4.rearrange("p (b c) -> p b c", c=CARD),
                    r1.unsqueeze(2).broadcast_to([128, 4, CARD]),
                    op=ALU.mult)

            def osb_transposes(g, pT, lv):
                """pT[0:32, half g] -> osb[:, bt, lv*32:(lv+1)*32]."""
                for bti in range(4):
                    bt = 4 * g + bti
                    trp = ps_sm.tile([128, P_IN], BF16, tag="ptr", bufs=1)
                    nc.tensor.transpose(
                        trp, pT[0:P_IN, bt * 128:(bt + 1) * 128], ident_sb)
                    nc.vector.tensor_copy(
                        osb[:, bt, lv * P_IN:(lv + 1) * P_IN], trp)

            def out_dma(g):
                o_r = out_d.ap().rearrange("p (t k) -> p t k", k=OUTW)
                nc.sync.dma_start(out=o_r[:, 4 * g:4 * g + 4],
                                  in_=osb[:, 4 * g:4 * g + 4])

            warm_i = [0]

            def warm(n):
                ps = ps_sm.tile([P_IN, 32], F32, tag="ptr", bufs=1,
                                name=f"warm{warm_i[0]}")
                warm_i[0] += 1
                for _ in range(n):
                    nc.tensor.matmul(ps, ident_sb, ident_sb,
                                     start=True, stop=True)

            # ================= emission schedule =================
            # PE order: enc(h0) | vals+scorer(h0) | sums r-h0 | enc(h1) |
            # mid-h/mid-logit/sums m-h0 | osb-transposes r-h0 |
            # vals+scorer(h1) | sums r-h1 | task(h0)+y | transposes m-h0 |
            # tail(h1) with warm filler.
            encoder_half(0)
            lg0 = ps_lg.tile([P_IN, 512], F32, tag="lgr", name="lg0")
            vals_scorer_half(0, lg0)
            softmax_chain(0, lg0, bsT_sb, 0, prT_sb)
            encoder_half(1)
            for q in range(4):
                mid_h_mms(0, q)
            ml0 = ps_lg.tile([P_IN, 512], F32, tag="lgm", name="ml0")
            mid_logit_mms(0, ml0)
            softmax_chain(0, ml0, b2cT_sb, 1, pmT_sb)
            osb_transposes(0, prT_sb, 0)

            lg1 = ps_lg.tile([P_IN, 512], F32, tag="lgr", name="lg1")
            vals_scorer_half(1, lg1)

            # h0 tail: task + y + output staging (hides under nothing on PE,
            # but all its PE ops are tiny; DVE/ACT run under vals h1)
            yl0 = ps_sm.tile([128, 16], F32, tag="ptr", bufs=1, name="yl0")
            task_mms(0, yl0)
            y_tail(0, yl0)
            osb_transposes(0, pmT_sb, 1)
            out_dma(0)

            # ---------------- h1 tail (end of kernel) ----------------
            softmax_chain(1, lg1, bsT_sb, 0, prT_sb)
            warm(8)
            for q in range(4):
                mid_h_mms(1, q)
            ml1 = ps_lg.tile([P_IN, 512], F32, tag="lgm", name="ml1")
            mid_logit_mms(1, ml1)
            softmax_chain(1, ml1, b2cT_sb, 1, pmT_sb)
            osb_transposes(1, prT_sb, 0)
            warm(8)
            yl1 = ps_sm.tile([128, 16], F32, tag="ptr", bufs=1, name="yl1")
            task_mms(1, yl1)
            warm(4)
            y_tail(1, yl1)
            osb_transposes(1, pmT_sb, 1)
            out_dma(1)

    nc.compile()
    return nc


def prep_weights(inp):
    """Host-side reformatting of (replicated) weights to device layouts."""
    f32 = np.float32
    fp8 = ml_dtypes.float8_e4m3

    def to_fp8(a):
        return np.clip(a, -240.0, 240.0).astype(fp8)

    W_enc = np.asarray(inp["W_enc"], f32)          # [2048, 1024]
    Wv = np.asarray(inp["Wv"], f32)                # [8, 1024, 256]
    Ws = np.asarray(inp["Ws"], f32)                # [8, 256, 4]
    W1c = np.asarray(inp["W1c"], f32)              # [8, 32, 64]
    W2c = np.asarray(inp["W2c"], f32)              # [8, 64, 4]
    W1y = np.asarray(inp["W1y"], f32)              # [32, 64]
    W2y = np.asarray(inp["W2y"], f32)              # [64, 4]
    b1c = np.asarray(inp["b1c"], f32)
    b1y = np.asarray(inp["b1y"], f32)
    b2y = np.asarray(inp["b2y"], f32)

    # wenc [p, ht, kt, c]
    wenc = (SW * W_enc).reshape(KT_IN, 128, KT_H, 128).transpose(1, 2, 0, 3)
    # wv [p, r, kt, oc]
    wv = (SW * Wv).reshape(N_ROOT, KT_H, 128, OV).transpose(2, 0, 1, 3)
    # ws_big [ch, 2r+ot, 4r+c]
    wsb = np.zeros((128, 16, 32), f32)
    for r in range(N_ROOT):
        for ot in range(2):
            wsb[:, 2 * r + ot, 4 * r:4 * r + 4] = \
                Ws[r, ot * 128:(ot + 1) * 128, :]
    # w1c pair layout [32, 4, 128] + b1c ones-row -> [33, 512]
    w1c_flat = W1c.transpose(1, 0, 2).reshape(P_IN, 512)
    w1c_aug = np.concatenate([w1c_flat, b1c.reshape(1, 512)], axis=0)
    # w2c_big [64s+h, q, 4(2q+s)+c]
    w2cb = np.zeros((128, 4, 32), f32)
    for q in range(4):
        for s in range(2):
            w2cb[64 * s:64 * s + 64, q, 4 * (2 * q + s):4 * (2 * q + s) + 4] \
                = W2c[2 * q + s]
    w1y_aug = np.concatenate([W1y, b1y.reshape(1, P_HID)], axis=0)
    w2y_aug = np.concatenate([W2y, b2y.reshape(1, CARD)], axis=0)
    # block-diagonal group-sum matrix
    g32 = np.kron(np.eye(8, dtype=f32), np.ones((4, 4), f32))

    bf16 = ml_dtypes.bfloat16
    # packed fp32 consts [128, 26]
    cstf = np.zeros((128, 26), f32)
    cstf[:, 0:8] = (SX * np.asarray(inp["b_enc"], f32)).reshape(KT_H, 128).T
    cstf[:, 8:24] = np.asarray(inp["bv"], f32).reshape(N_ROOT, 2, 128) \
        .transpose(2, 0, 1).reshape(128, 16)
    cstf[0:P_IN, 24] = np.asarray(inp["bs"], f32).reshape(P_IN)
    cstf[0:P_IN, 25] = np.asarray(inp["b2c"], f32).reshape(P_IN)
    # packed bf16 consts [128, 1284]
    cstb = np.zeros((128, 1284), f32)
    cstb[:, 0:512] = wsb.reshape(128, 512)
    cstb[:, 512:640] = w2cb.reshape(128, 128)
    cstb[0:P_IN + 1, 640:1152] = w1c_aug
    cstb[0:P_IN + 1, 1152:1216] = w1y_aug
    cstb[0:P_HID + 1, 1216:1220] = w2y_aug
    cstb[0:P_IN, 1220:1252] = np.eye(P_IN, dtype=f32)
    cstb[0:P_IN, 1252:1284] = g32
    wmap = {
        "wenc": np.ascontiguousarray(to_fp8(wenc).reshape(128, -1)),
        "wv": np.ascontiguousarray(to_fp8(wv).reshape(128, -1)),
        "cstf": np.ascontiguousarray(cstf),
        "cstb": np.ascontiguousarray(cstb, dtype=bf16),
    }
    return wmap


def make_in_maps(inp):
    f32 = np.float32
    fp8 = ml_dtypes.float8_e4m3
    bf16 = ml_dtypes.bfloat16
    wmap = prep_weights(inp)
    x = np.asarray(inp["x"], f32)
    lab = np.asarray(inp["c"], np.int32)
    msk = np.asarray(inp["intervention_index"], np.int32)

    # transposed one-hot / mask tensors, [4grp+c, b] per core
    iot = np.arange(CARD, dtype=np.int32)
    in_maps = []
    for i in range(N_CORES):
        m = dict(wmap)
        xc = x[i * BSH:(i + 1) * BSH]                     # [1024, 2048]
        xt = np.clip(SX * xc, -240.0, 240.0).astype(fp8)
        xt = xt.reshape(2, 512, KT_IN, 128).transpose(3, 0, 2, 1)
        m["xt"] = np.ascontiguousarray(xt.reshape(128, -1))
        lc = lab[i * BSH:(i + 1) * BSH]                   # [1024, 17]
        mc = msk[i * BSH:(i + 1) * BSH]
        # ohb[4r+c, b] = (lab[b, r] == c); cols 0:1024 root, 1024:2048 mid
        ohr = (lc[:, :8, None] == iot).transpose(1, 2, 0).reshape(P_IN, BSH)
        ohm = (lc[:, 8:16, None] == iot).transpose(1, 2, 0).reshape(P_IN, BSH)
        mrr = np.repeat(mc[:, :8].T, CARD, axis=0)        # [32, 1024]
        mmm = np.repeat(mc[:, 8:16].T, CARD, axis=0)
        m["ohb"] = np.ascontiguousarray(
            np.concatenate([ohr, ohm], axis=1).astype(bf16))
        m["mb"] = np.ascontiguousarray(
            np.concatenate([mrr, mmm], axis=1).astype(np.uint8))
        in_maps.append(m)
    return in_maps


def unshard_out(res_out):
    """[128, 8*68] per-core DRAM layout -> [BSH, 17, 4]."""
    a = np.asarray(res_out, np.float32).reshape(128, BSH // 128, 17, CARD)
    return np.ascontiguousarray(a.transpose(1, 0, 2, 3)).reshape(
        BSH, 17, CARD)


_NC_CACHE = {}


def _get_nc():
    key = SIM_SAFE
    if key not in _NC_CACHE:
        _NC_CACHE[key] = build_program()
    return _NC_CACHE[key]


def kernel(**inputs):
    from concourse.bass_utils import run_bass_kernel_spmd

    nc = _get_nc()
    in_maps = make_in_maps(inputs)
    res = run_bass_kernel_spmd(nc, in_maps, list(range(N_CORES)))
    outs = [unshard_out(res.results[i]["out"]) for i in range(N_CORES)]
    return np.concatenate(outs, axis=0)
